# revision 20
# baseline (speedup 1.0000x reference)
"""KAST scatter-memory kernel for Trainium2 (8 NeuronCores, data-parallel over batch).

Per core: one batch element, 15 sequential steps. Design:
  - base-2 softmax: k pre-scaled by sqrt(log2 e) on host, E = 2^(L2 - C2)
  - k-branch logits in f32r (exact); f32 kT via PE transposes in a dedicated
    1-bank PSUM staging area, emitted as filler work between logit tiles
  - m-branch fully bf16: kT via DMA-transpose of host-supplied bf16 k,
    m_k EMA in bf16 on DVE (2x mode)
  - exp split: most kk-tiles on ACT (Exp with scale=ln2, bias=-C2*ln2 -> bf16 E),
    CHAIN_T tiles via DVE shift (L2-C2) + gpsimd pow(2, .) to offload ACT
  - rec reduction reoriented: out [128q, 4] per (sim, q-tile), E tile stationary
    -> near-zero PE cost, naturally-laid-out output; interleaved as PE filler
  - gates: host-computed sigmoid (no per-step ACT table swaps)
"""
import sys

sys.path.insert(0, "/opt/trn_rl_repo")

import numpy as np
import ml_dtypes

import concourse.bass as bass
import concourse.tile as tile
from concourse import bacc, mybir
from concourse.bass_utils import run_bass_kernel_spmd
from concourse.masks import make_identity

F32 = mybir.dt.float32
F32R = mybir.dt.float32r
BF16 = mybir.dt.bfloat16
AF = mybir.ActivationFunctionType
ALU = mybir.AluOpType

BS, SEQ, H, W, CK = 8, 16, 32, 32, 256
HW = H * W          # 1024
CV = 3
NT = HW // 128      # 8 hw tiles
NC2 = CK // 128     # 2 ck chunks
LN2 = 0.6931471805599453
SQRT_LOG2E = 1.2011224087864498   # k pre-scale: L2 = log2(e) * L
C2 = 80.0                         # base-2 shift: scaled logits <= ~196, rowmax >= ~33
CHAIN_T = (0, 4)                  # kk-tiles exp'd via DVE shift + gpsimd pow (per sim)
COEF = 0.1

_CACHE = {}


def _r(x):
    return x.bitcast(F32R)


def build_program():
    nc = bacc.Bacc("TRN2", target_bir_lowering=False, debug=False, num_devices=8)

    k_d = nc.dram_tensor("k", [2, HW, CK], F32, kind="ExternalInput")  # frames 0-1 f32
    kb_d = nc.dram_tensor("kb", [SEQ, HW, CK], BF16, kind="ExternalInput")  # bf16 hi
    kl_d = nc.dram_tensor("kl", [SEQ, HW, CK], BF16, kind="ExternalInput")  # bf16 lo
    v_d = nc.dram_tensor("v", [SEQ, HW, CV], F32, kind="ExternalInput")
    g_d = nc.dram_tensor("gate", [SEQ, HW], BF16, kind="ExternalInput")  # sigmoid(att)
    gn_d = nc.dram_tensor("gnat", [128, SEQ - 1, NT], F32, kind="ExternalInput")
    m_d = nc.dram_tensor("maskf", [1, SEQ], F32, kind="ExternalInput")
    o_d = nc.dram_tensor("out_v", [SEQ - 1, HW, CV], F32, kind="ExternalOutput")

    with tile.TileContext(nc) as tc:
        with (
            tc.tile_pool(name="persist", bufs=1) as P1,
            tc.tile_pool(name="kn", bufs=4) as PKN,
            tc.tile_pool(name="knb", bufs=2) as PKNB,
            tc.tile_pool(name="kt", bufs=3) as PKT,
            tc.tile_pool(name="ktm", bufs=3) as PKTM,
            tc.tile_pool(name="ktl", bufs=2) as PKTL,
            tc.tile_pool(name="ema", bufs=2) as PEMA,
            tc.tile_pool(name="g", bufs=2) as PG,
            tc.tile_pool(name="sh", bufs=4) as PSH,
            tc.tile_pool(name="ek", bufs=17) as PEK,
            tc.tile_pool(name="em", bufs=17) as PEM,
            tc.tile_pool(name="small", bufs=3) as PSM,
            tc.tile_pool(name="psL", bufs=3, space="PSUM") as PSL,
            tc.tile_pool(name="psT", bufs=1, space="PSUM") as PST,
            tc.tile_pool(name="psR", bufs=1, space="PSUM") as PSR,
        ):
            def transp_load(f):
                """DMAs for frame f. Frames 0-1: f32 kn halves (PE transposes).
                Frames 2+: bf16 lo + DMA-transpose (kT = hi + lo on DVE later).
                Returns (aux, kT tile to fill, kTm)."""
                knb = PKNB.tile([128, NT, CK], BF16, tag="knb", name=f"knb{f}")
                nc.sync.dma_start(
                    out=knb, in_=kb_d[f].rearrange("(t p) c -> p t c", p=128)
                )
                # [128, 2t+s, 128]: kTm[p, 2t+s, i] = k_hi[t*128+i, s*128+p]
                kTm = PKTM.tile([128, 2 * NT, 128], BF16, tag="kTm", name=f"kTm{f}")
                nc.sync.dma_start_transpose(
                    out=kTm, in_=knb.rearrange("p t c -> p (t c)")
                )
                kT = PKT.tile([128, NC2, HW], F32, tag="kT", name=f"kT{f}")
                if f < 2:
                    knh = []
                    for c in range(NC2):
                        kn = PKN.tile([128, NT, 128], F32, tag=f"kn{c}", name=f"kn{f}_{c}")
                        nc.sync.dma_start(
                            out=kn,
                            in_=k_d[f, :, c * 128 : (c + 1) * 128].rearrange(
                                "(t p) c -> p t c", p=128
                            ),
                        )
                        knh.append(kn)
                    return knh, kT, kTm
                knl = PKNB.tile([128, NT, CK], BF16, tag="knl", name=f"knl{f}")
                nc.sync.dma_start(
                    out=knl, in_=kl_d[f].rearrange("(t p) c -> p t c", p=128)
                )
                kTl = PKTL.tile([128, 2 * NT, 128], BF16, tag="kTl", name=f"kTl{f}")
                nc.sync.dma_start_transpose(
                    out=kTl, in_=knl.rearrange("p t c -> p (t c)")
                )
                return kTl, kTm, kT, kTm  # type: ignore[return-value]

            def kt_build(kTm, kTl, kT):
                """f32 kT[:, s, (t i)] = hi + lo (Pool adds, f32r-rounded)."""
                for s in range(NC2):
                    nc.gpsimd.tensor_add(
                        kT[:, s, :].rearrange("p (t i) -> p t i", i=128).bitcast(F32R),
                        kTm[:, s : 2 * NT : 2, :],
                        kTl[:, s : 2 * NT : 2, :],
                    )

            def tq_emit(f, knh, kT, q):
                """PE-transpose quarter q (c=q//2, half=q%2) of frame f into kT."""
                c, h = q // 2, q % 2
                ps = PST.tile([128, 512], F32, tag="pst", name=f"pst{f}_{q}")
                for tq in range(4):
                    nc.tensor.transpose(
                        out=ps[:, tq * 128 : (tq + 1) * 128],
                        in_=knh[c][:, h * 4 + tq, :],
                        identity=ident,
                    )
                nc.vector.tensor_copy(
                    out=_r(kT[:, c, h * 512 : (h + 1) * 512]), in_=ps
                )

            # --- early input DMAs for the first two frames
            lm0 = transp_load(0)
            lm1 = transp_load(1)

            ident = P1.tile([128, 128], F32)
            make_identity(nc, ident)
            negC2 = P1.tile([128, 1], F32)
            nc.vector.memset(negC2, -C2 * LN2)
            two = P1.tile([128, 1], F32)
            nc.vector.memset(two, 2.0)

            # persistent state
            m_kT = P1.tile([128, NC2, HW], BF16, tag="mkT")
            nc.gpsimd.memset(m_kT, 0.0)
            mv4 = P1.tile([128, 4 * NT], F32, tag="mv4")
            nc.vector.memset(mv4, 0.0)
            nc.vector.memset(mv4[:, 3 : 4 * NT : 4], 1.0)

            mask_bc = P1.tile([128, SEQ], F32, tag="maskbc")
            nc.sync.dma_start(out=mask_bc, in_=m_d[0:1, :].partition_broadcast(128))
            gnat = P1.tile([128, SEQ - 1, NT], F32, tag="gnat")
            nc.sync.dma_start(out=gnat, in_=gn_d[:, :, :])

            def load_v1(f):
                t = PSM.tile([128, NT, 4], F32, tag="v1")
                nc.vector.memset(t[:, :, 3:4], 1.0)
                nc.sync.dma_start(
                    out=t[:, :, 0:CV],
                    in_=v_d[f].rearrange("(t p) c -> p t c", p=128),
                )
                return t.rearrange("p t c -> p (t c)")

            # prologue: build kT(0), kT(1) via PE transposes
            for q in range(4):
                tq_emit(0, lm0[0], lm0[1], q)
            for q in range(4):
                tq_emit(1, lm1[0], lm1[1], q)
            kT_i, kTm_i = lm0[1], lm0[2]
            kT_n, kTm_n = lm1[1], lm1[2]

            pv4raw = load_v1(0)
            pv4 = PSM.tile([128, 4 * NT], F32, tag="pv4", name="pv4_init")
            nc.vector.tensor_copy(out=_r(pv4), in_=pv4raw)
            pv4b = PSM.tile([128, 4 * NT], BF16, tag="pv4b", name="pv4b_init")
            nc.vector.tensor_copy(out=pv4b, in_=pv4)

            def logits_exp(i, sim, kT_st, kT_q, kTm_q, fillers):
                """Logits matmuls + exp for one sim; one filler popped per tile."""
                pool = PEK if sim == "k" else PEM
                E = [pool.tile([128, HW], BF16, tag="e" + sim, name=f"e{sim}{i}_{t}") for t in range(NT)]
                for t in range(NT):
                    ps = PSL.tile([128, HW], F32, tag="psl", name=f"psl{i}_{sim}{t}")
                    for c in range(NC2):
                        for hh in range(2):
                            if sim == "k":
                                nc.tensor.matmul(
                                    ps[:, hh * 512 : (hh + 1) * 512],
                                    _r(kT_st[:, c, t * 128 : (t + 1) * 128]),
                                    _r(kT_q[:, c, hh * 512 : (hh + 1) * 512]),
                                    start=(c == 0),
                                    stop=(c == NC2 - 1),
                                )
                            else:
                                nc.tensor.matmul(
                                    ps[:, hh * 512 : (hh + 1) * 512],
                                    m_kT[:, c, t * 128 : (t + 1) * 128],
                                    kTm_q[:, 8 * hh + c : 8 * hh + 8 : 2, :],
                                    start=(c == 0),
                                    stop=(c == NC2 - 1),
                                )
                    chain = t in CHAIN_T or (
                        i == SEQ - 2 and sim == "m" and t in (6, 7)
                    )
                    if chain:
                        sh = PSH.tile([128, HW], F32, tag="sh")
                        nc.vector.tensor_scalar_add(sh, ps, -C2)
                        nc.gpsimd.tensor_tensor(
                            out=E[t],
                            in0=two.broadcast_to([128, HW]),
                            in1=sh,
                            op=ALU.pow,
                        )
                    else:
                        nc.scalar.activation(
                            E[t], ps, AF.Exp, bias=negC2[:, 0:1], scale=LN2
                        )
                    if fillers:
                        fillers.pop(0)()
                return E

            def rec_chunks(j, E_k, E_m, pv4b_j, mv4b_j):
                """Return (recPS, 16 filler callables) for step j's reduction."""
                recPS = PSR.tile([128, 512], F32, tag="rec", name=f"rec{j}")
                fillers = []
                for s, (E, rhs4) in enumerate(((E_k, pv4b_j), (E_m, mv4b_j))):
                    for qt in range(NT):
                        def chunk(s=s, qt=qt, E=E, rhs4=rhs4):
                            for t in range(NT):
                                nc.tensor.matmul(
                                    recPS[:, s * 32 + qt * 4 : s * 32 + (qt + 1) * 4],
                                    E[t][:, qt * 128 : (qt + 1) * 128],
                                    rhs4[:, t * 4 : (t + 1) * 4],
                                    start=(t == 0),
                                    stop=(t == NT - 1),
                                    skip_group_check=True,
                                )
                        fillers.append(chunk)
                return recPS, fillers

            def blend_out(j, recPS):
                """rec = 0.9*Nk/Dk + 0.1*Nm/Dm from the accumulated bank; DMA out."""
                Nnat = PSM.tile([128, 64], F32, tag="Nnat")
                nc.vector.tensor_copy(out=Nnat, in_=recPS[:, 0:64])
                rD = PSM.tile([128, 16], F32, tag="rD")
                nc.vector.reciprocal(rD, Nnat[:, 3:64:4])
                nc.vector.tensor_scalar_mul(rD[:, 0:8], rD[:, 0:8], 1.0 - COEF)
                nc.vector.tensor_scalar_mul(rD[:, 8:16], rD[:, 8:16], COEF)
                rDe = PSM.tile([128, 16, 4], F32, tag="rDe")
                nc.vector.tensor_copy(
                    out=rDe, in_=rD.unsqueeze(-1).broadcast_to([128, 16, 4])
                )
                rDe = rDe.rearrange("p t c -> p (t c)")
                Ns = PSM.tile([128, 64], F32, tag="Ns")
                nc.vector.tensor_mul(Ns, Nnat, rDe)
                rec = PSM.tile([128, 32], F32, tag="recn")
                nc.vector.tensor_add(rec, Ns[:, 0:32], Ns[:, 32:64])
                nc.sync.dma_start(
                    out=o_d[j].rearrange("(t p) c -> p t c", p=128),
                    in_=rec.rearrange("p (t c) -> p t c", c=4)[:, :, 0:CV],
                )
                return rec

            prev = None  # (j, E_k, E_m, pv4b_j, mv4b_j) awaiting reduction
            for i in range(SEQ - 1):
                if i + 2 <= SEQ - 1:
                    r = transp_load(i + 2)
                    kTl2, kT2, kTm_n2 = r[0], r[2], r[3]
                else:
                    kTl2, kT2, kTm_n2 = None, None, None

                # --- bf16 gate broadcast + m_kT EMA
                G = PG.tile([128, HW], BF16, tag="G")
                nc.sync.dma_start(
                    out=G, in_=g_d[i : i + 1, :].partition_broadcast(128)
                )
                for s in range(NC2):
                    ksv = kTm_i[:, s : 2 * NT : 2, :]                       # [128, 8t, 128]
                    msv = m_kT[:, s, :].rearrange("p (t i) -> p t i", i=128)
                    gsv = G.rearrange("p (t i) -> p t i", i=128)
                    tmp = PEMA.tile([128, NT, 128], BF16, tag=f"ema{s}")
                    nc.vector.tensor_sub(tmp, ksv, msv)
                    nc.vector.tensor_mul(tmp, tmp, gsv)
                    nc.vector.tensor_add(msv.bitcast(BF16), msv, tmp)

                # --- fillers: prev-step rec chunks + next-frame transposes
                if prev is not None:
                    j, pE_k, pE_m, ppv4b, pmv4b = prev
                    recPS, fillers = rec_chunks(j, pE_k, pE_m, ppv4b, pmv4b)
                else:
                    recPS, fillers = None, []

                # --- logits + exp for both sims, fillers interleaved
                E_k = logits_exp(i, "k", kT_i, kT_n, kTm_n, fillers)
                # build f32 kT(i+2) = hi + lo on Pool (DMA-transposes long done)
                if kT2 is not None:
                    kt_build(kTm_n2, kTl2, kT2)
                E_m = logits_exp(i, "m", kT_i, kT_n, kTm_n, fillers)
                for f in fillers:
                    f()

                # --- blend + output of step i-1 -> pv4(i)
                if prev is not None:
                    rec = blend_out(j, recPS)
                    v1 = load_v1(j)
                    diff = PSM.tile([128, 32], F32, tag="diff")
                    nc.vector.tensor_sub(diff, v1, rec)
                    nc.vector.tensor_scalar_mul(diff, diff, mask_bc[:, j : j + 1])
                    pv4_new = PSM.tile([128, 32], F32, tag="pv4")
                    nc.vector.tensor_add(_r(pv4_new), rec, diff)
                    pv4 = pv4_new
                    pv4b = PSM.tile([128, 32], BF16, tag="pv4b")
                    nc.vector.tensor_copy(out=pv4b, in_=pv4)

                # --- m_v EMA (pv of step i; only needed by recT(i) next iter)
                gb32 = PSM.tile([128, NT, 4], F32, tag="gb32")
                nc.vector.tensor_copy(
                    out=gb32,
                    in_=gnat[:, i, :].unsqueeze(-1).broadcast_to([128, NT, 4]),
                )
                gb32 = gb32.rearrange("p t c -> p (t c)")
                tmpv = PSM.tile([128, 4 * NT], F32, tag="tmpv")
                nc.vector.tensor_sub(tmpv, pv4, mv4)
                nc.vector.tensor_mul(tmpv, tmpv, gb32)
                nc.vector.tensor_add(_r(mv4), mv4, tmpv)
                mv4b = PSM.tile([128, 4 * NT], BF16, tag="mv4b")
                nc.vector.tensor_copy(out=mv4b, in_=mv4)

                prev = (i, E_k, E_m, pv4b, mv4b)
                if i < SEQ - 2:
                    kT_i, kTm_i = kT_n, kTm_n
                    kT_n, kTm_n = kT2, kTm_n2

            # epilogue: reduce the final step
            j, pE_k, pE_m, ppv4b, pmv4b = prev
            recPS, fillers = rec_chunks(j, pE_k, pE_m, ppv4b, pmv4b)
            for f in fillers:
                f()
            blend_out(j, recPS)

    nc.compile()
    return nc


def make_in_maps(k, v, attention, seq_mask):
    k = np.asarray(k, dtype=np.float32)
    v = np.ascontiguousarray(np.asarray(v, dtype=np.float32))
    attention = np.asarray(attention, dtype=np.float32)
    seq_mask = np.asarray(seq_mask)

    ks = np.ascontiguousarray(k * np.float32(SQRT_LOG2E))
    kb = np.ascontiguousarray(ks.astype(ml_dtypes.bfloat16))
    kl = np.ascontiguousarray((ks - kb.astype(np.float32)).astype(ml_dtypes.bfloat16))
    gate64 = 1.0 / (1.0 + np.exp(-attention, dtype=np.float64))
    gate = gate64.astype(np.float32)
    gate_b = np.ascontiguousarray(gate.astype(ml_dtypes.bfloat16))
    gnat = np.ascontiguousarray(
        gate.reshape(BS, SEQ, HW)[:, : SEQ - 1, :]
        .reshape(BS, SEQ - 1, NT, 128)
        .transpose(0, 3, 1, 2)
    )
    maskf = seq_mask.astype(np.float32)

    in_maps = []
    for b in range(BS):
        in_maps.append(
            {
                "k": ks[b].reshape(SEQ, HW, CK)[0:2],
                "kb": kb[b].reshape(SEQ, HW, CK),
                "kl": kl[b].reshape(SEQ, HW, CK),
                "v": v[b].reshape(SEQ, HW, CV),
                "gate": gate_b[b].reshape(SEQ, HW),
                "gnat": gnat[b],
                "maskf": np.ascontiguousarray(maskf[b : b + 1]),
            }
        )
    return in_maps


def kernel(k, v, attention, seq_mask):
    v = np.ascontiguousarray(np.asarray(v, dtype=np.float32))

    if "nc" not in _CACHE:
        _CACHE["nc"] = build_program()
    nc = _CACHE["nc"]

    in_maps = make_in_maps(k, v, attention, seq_mask)
    res = run_bass_kernel_spmd(nc, in_maps, list(range(BS)))
    out_v = np.stack([res.results[b]["out_v"] for b in range(BS)]).reshape(
        BS, SEQ - 1, H, W, CV
    )
    gt = v[:, 1:].reshape(BS, SEQ - 1, H, W, CV)
    return out_v, gt


# revision 24
# speedup vs baseline: 1.0825x; 1.0825x over previous
"""KAST scatter-memory kernel for Trainium2 (8 NeuronCores, data-parallel over batch).

Per core: one batch element, 15 sequential steps. Design:
  - base-2 softmax: k pre-scaled by sqrt(log2 e) on host, E = 2^(L2 - C2)
  - k-branch logits in f32r (exact); f32 kT via PE transposes in a dedicated
    1-bank PSUM staging area, emitted as filler work between logit tiles
  - m-branch fully bf16: kT via DMA-transpose of host-supplied bf16 k,
    m_k EMA in bf16 on DVE (2x mode)
  - exp split: most kk-tiles on ACT (Exp with scale=ln2, bias=-C2*ln2 -> bf16 E),
    CHAIN_T tiles via DVE shift (L2-C2) + gpsimd pow(2, .) to offload ACT
  - rec reduction reoriented: out [128q, 4] per (sim, q-tile), E tile stationary
    -> near-zero PE cost, naturally-laid-out output; interleaved as PE filler
  - gates: host-computed sigmoid (no per-step ACT table swaps)
"""
import sys

sys.path.insert(0, "/opt/trn_rl_repo")

import numpy as np
import ml_dtypes

import concourse.bass as bass
import concourse.tile as tile
from concourse import bacc, mybir
from concourse.bass_utils import run_bass_kernel_spmd
from concourse.masks import make_identity

F32 = mybir.dt.float32
F32R = mybir.dt.float32r
BF16 = mybir.dt.bfloat16
AF = mybir.ActivationFunctionType
ALU = mybir.AluOpType

BS, SEQ, H, W, CK = 8, 16, 32, 32, 256
HW = H * W          # 1024
CV = 3
NT = HW // 128      # 8 hw tiles
NC2 = CK // 128     # 2 ck chunks
LN2 = 0.6931471805599453
SQRT_LOG2E = 1.2011224087864498   # k pre-scale: L2 = log2(e) * L
C2 = 80.0                         # base-2 shift: scaled logits <= ~196, rowmax >= ~33
CHAIN_T = (0, 4)                  # kk-tiles exp'd via DVE shift + gpsimd pow (per sim)
COEF = 0.1

_CACHE = {}


def _r(x):
    return x.bitcast(F32R)


def build_program():
    nc = bacc.Bacc("TRN2", target_bir_lowering=False, debug=False, num_devices=8)

    k_d = nc.dram_tensor("k", [SEQ, HW, CK], F32, kind="ExternalInput")  # * sqrt(log2 e)
    kb_d = nc.dram_tensor("kb", [SEQ, HW, CK], BF16, kind="ExternalInput")  # bf16 copy
    v_d = nc.dram_tensor("v", [SEQ, HW, CV], F32, kind="ExternalInput")
    g_d = nc.dram_tensor("gate", [SEQ, HW], BF16, kind="ExternalInput")  # sigmoid(att)
    gn_d = nc.dram_tensor("gnat", [128, SEQ - 1, NT], F32, kind="ExternalInput")
    m_d = nc.dram_tensor("maskf", [1, SEQ], F32, kind="ExternalInput")
    o_d = nc.dram_tensor("out_v", [SEQ - 1, HW, CV], F32, kind="ExternalOutput")

    with tile.TileContext(nc) as tc:
        with (
            tc.tile_pool(name="persist", bufs=1) as P1,
            tc.tile_pool(name="kn", bufs=4) as PKN,
            tc.tile_pool(name="knb", bufs=2) as PKNB,
            tc.tile_pool(name="kt", bufs=3) as PKT,
            tc.tile_pool(name="ktm", bufs=3) as PKTM,
            tc.tile_pool(name="ema", bufs=2) as PEMA,
            tc.tile_pool(name="g", bufs=2) as PG,
            tc.tile_pool(name="sh", bufs=4) as PSH,
            tc.tile_pool(name="ek", bufs=17) as PEK,
            tc.tile_pool(name="em", bufs=17) as PEM,
            tc.tile_pool(name="small", bufs=3) as PSM,
            tc.tile_pool(name="psL", bufs=3, space="PSUM") as PSL,
            tc.tile_pool(name="psT", bufs=1, space="PSUM") as PST,
            tc.tile_pool(name="psR", bufs=1, space="PSUM") as PSR,
        ):
            def transp_load(f):
                """DMAs for frame f: f32 kn in 2 ck-halves + bf16 DMA-transpose.
                Returns (kn_halves, kT tile to fill, kTm)."""
                knh = []
                for c in range(NC2):
                    kn = PKN.tile([128, NT, 128], F32, tag=f"kn{c}", name=f"kn{f}_{c}")
                    nc.sync.dma_start(
                        out=kn,
                        in_=k_d[f, :, c * 128 : (c + 1) * 128].rearrange(
                            "(t p) c -> p t c", p=128
                        ),
                    )
                    knh.append(kn)
                knb = PKNB.tile([128, NT, CK], BF16, tag="knb", name=f"knb{f}")
                nc.sync.dma_start(
                    out=knb, in_=kb_d[f].rearrange("(t p) c -> p t c", p=128)
                )
                # [128, 2t+s, 128]: kTm[p, 2t+s, i] = k_bf[t*128+i, s*128+p]
                kTm = PKTM.tile([128, 2 * NT, 128], BF16, tag="kTm", name=f"kTm{f}")
                nc.sync.dma_start_transpose(
                    out=kTm, in_=knb.rearrange("p t c -> p (t c)")
                )
                kT = PKT.tile([128, NC2, HW], F32, tag="kT", name=f"kT{f}")
                return knh, kT, kTm

            def tq_emit(f, knh, kT, q):
                """PE-transpose quarter q (c=q//2, half=q%2) of frame f into kT."""
                c, h = q // 2, q % 2
                ps = PST.tile([128, 512], F32, tag="pst", name=f"pst{f}_{q}")
                for tq in range(4):
                    nc.tensor.transpose(
                        out=ps[:, tq * 128 : (tq + 1) * 128],
                        in_=knh[c][:, h * 4 + tq, :],
                        identity=ident,
                    )
                nc.vector.tensor_copy(
                    out=_r(kT[:, c, h * 512 : (h + 1) * 512]), in_=ps
                )

            # --- early input DMAs for the first two frames
            lm0 = transp_load(0)
            lm1 = transp_load(1)

            ident = P1.tile([128, 128], F32)
            make_identity(nc, ident)
            negC2 = P1.tile([128, 1], F32)
            nc.vector.memset(negC2, -C2 * LN2)
            two = P1.tile([128, 1], F32)
            nc.vector.memset(two, 2.0)

            # persistent state
            m_kT = P1.tile([128, NC2, HW], BF16, tag="mkT")
            nc.gpsimd.memset(m_kT, 0.0)
            mv4 = P1.tile([128, 4 * NT], F32, tag="mv4")
            nc.vector.memset(mv4, 0.0)
            nc.vector.memset(mv4[:, 3 : 4 * NT : 4], 1.0)

            mask_bc = P1.tile([128, SEQ], F32, tag="maskbc")
            nc.sync.dma_start(out=mask_bc, in_=m_d[0:1, :].partition_broadcast(128))
            gnat = P1.tile([128, SEQ - 1, NT], F32, tag="gnat")
            nc.sync.dma_start(out=gnat, in_=gn_d[:, :, :])

            def load_v1(f):
                t = PSM.tile([128, NT, 4], F32, tag="v1")
                nc.vector.memset(t[:, :, 3:4], 1.0)
                nc.sync.dma_start(
                    out=t[:, :, 0:CV],
                    in_=v_d[f].rearrange("(t p) c -> p t c", p=128),
                )
                return t.rearrange("p t c -> p (t c)")

            # prologue: build kT(0), kT(1) immediately
            for q in range(4):
                tq_emit(0, lm0[0], lm0[1], q)
            for q in range(4):
                tq_emit(1, lm1[0], lm1[1], q)
            kT_i, kTm_i = lm0[1], lm0[2]
            kT_n, kTm_n = lm1[1], lm1[2]

            pv4raw = load_v1(0)
            pv4 = PSM.tile([128, 4 * NT], F32, tag="pv4", name="pv4_init")
            nc.vector.tensor_copy(out=_r(pv4), in_=pv4raw)
            pv4b = PSM.tile([128, 4 * NT], BF16, tag="pv4b", name="pv4b_init")
            nc.vector.tensor_copy(out=pv4b, in_=pv4)

            def logits_exp(i, sim, kT_st, kT_q, kTm_q, fillers):
                """Logits matmuls + exp for one sim; one filler popped per tile."""
                pool = PEK if sim == "k" else PEM
                E = [pool.tile([128, HW], BF16, tag="e" + sim, name=f"e{sim}{i}_{t}") for t in range(NT)]
                for t in range(NT):
                    ps = PSL.tile([128, HW], F32, tag="psl", name=f"psl{i}_{sim}{t}")
                    for c in range(NC2):
                        for hh in range(2):
                            if sim == "k":
                                nc.tensor.matmul(
                                    ps[:, hh * 512 : (hh + 1) * 512],
                                    _r(kT_st[:, c, t * 128 : (t + 1) * 128]),
                                    _r(kT_q[:, c, hh * 512 : (hh + 1) * 512]),
                                    start=(c == 0),
                                    stop=(c == NC2 - 1),
                                )
                            else:
                                nc.tensor.matmul(
                                    ps[:, hh * 512 : (hh + 1) * 512],
                                    m_kT[:, c, t * 128 : (t + 1) * 128],
                                    kTm_q[:, 8 * hh + c : 8 * hh + 8 : 2, :],
                                    start=(c == 0),
                                    stop=(c == NC2 - 1),
                                )
                    chain = t in CHAIN_T or (
                        i == SEQ - 2 and sim == "m" and t in (5, 6, 7)
                    )
                    if chain:
                        sh = PSH.tile([128, HW], F32, tag="sh")
                        nc.vector.tensor_scalar_add(sh, ps, -C2)
                        nc.gpsimd.tensor_tensor(
                            out=E[t],
                            in0=two.broadcast_to([128, HW]),
                            in1=sh,
                            op=ALU.pow,
                        )
                    else:
                        nc.scalar.activation(
                            E[t], ps, AF.Exp, bias=negC2[:, 0:1], scale=LN2
                        )
                    if fillers:
                        fillers.pop(0)()
                return E

            def rec_chunks(j, E_k, E_m, pv4b_j, mv4b_j):
                """Return (recPS, 16 filler callables) for step j's reduction."""
                recPS = PSR.tile([128, 512], F32, tag="rec", name=f"rec{j}")
                fillers = []
                for s, (E, rhs4) in enumerate(((E_k, pv4b_j), (E_m, mv4b_j))):
                    for qt in range(NT):
                        def chunk(s=s, qt=qt, E=E, rhs4=rhs4):
                            for t in range(NT):
                                nc.tensor.matmul(
                                    recPS[:, s * 32 + qt * 4 : s * 32 + (qt + 1) * 4],
                                    E[t][:, qt * 128 : (qt + 1) * 128],
                                    rhs4[:, t * 4 : (t + 1) * 4],
                                    start=(t == 0),
                                    stop=(t == NT - 1),
                                    skip_group_check=True,
                                )
                        fillers.append(chunk)
                return recPS, fillers

            def blend_out(j, recPS):
                """rec = 0.9*Nk/Dk + 0.1*Nm/Dm from the accumulated bank; DMA out."""
                Nnat = PSM.tile([128, 64], F32, tag="Nnat")
                nc.vector.tensor_copy(out=Nnat, in_=recPS[:, 0:64])
                rD = PSM.tile([128, 16], F32, tag="rD")
                nc.vector.reciprocal(rD, Nnat[:, 3:64:4])
                nc.vector.tensor_scalar_mul(rD[:, 0:8], rD[:, 0:8], 1.0 - COEF)
                nc.vector.tensor_scalar_mul(rD[:, 8:16], rD[:, 8:16], COEF)
                rDe = PSM.tile([128, 16, 4], F32, tag="rDe")
                nc.vector.tensor_copy(
                    out=rDe, in_=rD.unsqueeze(-1).broadcast_to([128, 16, 4])
                )
                rDe = rDe.rearrange("p t c -> p (t c)")
                Ns = PSM.tile([128, 64], F32, tag="Ns")
                nc.vector.tensor_mul(Ns, Nnat, rDe)
                rec = PSM.tile([128, 32], F32, tag="recn")
                nc.vector.tensor_add(rec, Ns[:, 0:32], Ns[:, 32:64])
                nc.sync.dma_start(
                    out=o_d[j].rearrange("(t p) c -> p t c", p=128),
                    in_=rec.rearrange("p (t c) -> p t c", c=4)[:, :, 0:CV],
                )
                return rec

            prev = None  # (j, E_k, E_m, pv4b_j, mv4b_j) awaiting reduction
            for i in range(SEQ - 1):
                if i + 2 <= SEQ - 1:
                    kn2h, kT2, kTm_n2 = transp_load(i + 2)
                else:
                    kn2h, kT2, kTm_n2 = None, None, None

                # --- bf16 gate broadcast + m_kT EMA
                G = PG.tile([128, HW], BF16, tag="G")
                nc.sync.dma_start(
                    out=G, in_=g_d[i : i + 1, :].partition_broadcast(128)
                )
                for s in range(NC2):
                    ksv = kTm_i[:, s : 2 * NT : 2, :]                       # [128, 8t, 128]
                    msv = m_kT[:, s, :].rearrange("p (t i) -> p t i", i=128)
                    gsv = G.rearrange("p (t i) -> p t i", i=128)
                    tmp = PEMA.tile([128, NT, 128], BF16, tag=f"ema{s}")
                    nc.vector.tensor_sub(tmp, ksv, msv)
                    nc.vector.tensor_mul(tmp, tmp, gsv)
                    nc.vector.tensor_add(msv.bitcast(BF16), msv, tmp)

                # --- fillers: prev-step rec chunks + next-frame transposes
                if prev is not None:
                    j, pE_k, pE_m, ppv4b, pmv4b = prev
                    recPS, fillers = rec_chunks(j, pE_k, pE_m, ppv4b, pmv4b)
                else:
                    recPS, fillers = None, []
                if kn2h is not None:
                    for qi, pos in enumerate((2, 6, 10, 14)):
                        tq = (lambda q: lambda: tq_emit(i + 2, kn2h, kT2, q))(qi)
                        if pos < len(fillers):
                            fillers.insert(pos, tq)
                        else:
                            fillers.append(tq)

                # --- logits + exp for both sims, fillers interleaved
                E_k = logits_exp(i, "k", kT_i, kT_n, kTm_n, fillers)
                E_m = logits_exp(i, "m", kT_i, kT_n, kTm_n, fillers)
                for f in fillers:
                    f()

                # --- blend + output of step i-1 -> pv4(i)
                if prev is not None:
                    rec = blend_out(j, recPS)
                    v1 = load_v1(j)
                    diff = PSM.tile([128, 32], F32, tag="diff")
                    nc.vector.tensor_sub(diff, v1, rec)
                    nc.vector.tensor_scalar_mul(diff, diff, mask_bc[:, j : j + 1])
                    pv4_new = PSM.tile([128, 32], F32, tag="pv4")
                    nc.vector.tensor_add(_r(pv4_new), rec, diff)
                    pv4 = pv4_new
                    pv4b = PSM.tile([128, 32], BF16, tag="pv4b")
                    nc.vector.tensor_copy(out=pv4b, in_=pv4)

                # --- m_v EMA (pv of step i; only needed by recT(i) next iter)
                gb32 = PSM.tile([128, NT, 4], F32, tag="gb32")
                nc.vector.tensor_copy(
                    out=gb32,
                    in_=gnat[:, i, :].unsqueeze(-1).broadcast_to([128, NT, 4]),
                )
                gb32 = gb32.rearrange("p t c -> p (t c)")
                tmpv = PSM.tile([128, 4 * NT], F32, tag="tmpv")
                nc.vector.tensor_sub(tmpv, pv4, mv4)
                nc.vector.tensor_mul(tmpv, tmpv, gb32)
                nc.vector.tensor_add(_r(mv4), mv4, tmpv)
                mv4b = PSM.tile([128, 4 * NT], BF16, tag="mv4b")
                nc.vector.tensor_copy(out=mv4b, in_=mv4)

                prev = (i, E_k, E_m, pv4b, mv4b)
                if i < SEQ - 2:
                    kT_i, kTm_i = kT_n, kTm_n
                    kT_n, kTm_n = kT2, kTm_n2

            # epilogue: reduce the final step
            j, pE_k, pE_m, ppv4b, pmv4b = prev
            recPS, fillers = rec_chunks(j, pE_k, pE_m, ppv4b, pmv4b)
            for f in fillers:
                f()
            blend_out(j, recPS)

    nc.compile()
    return nc


def make_in_maps(k, v, attention, seq_mask):
    k = np.asarray(k, dtype=np.float32)
    v = np.ascontiguousarray(np.asarray(v, dtype=np.float32))
    attention = np.asarray(attention, dtype=np.float32)
    seq_mask = np.asarray(seq_mask)

    ks = np.ascontiguousarray(k * np.float32(SQRT_LOG2E))
    kb = np.ascontiguousarray(ks.astype(ml_dtypes.bfloat16))
    gate64 = 1.0 / (1.0 + np.exp(-attention, dtype=np.float64))
    gate = gate64.astype(np.float32)
    gate_b = np.ascontiguousarray(gate.astype(ml_dtypes.bfloat16))
    gnat = np.ascontiguousarray(
        gate.reshape(BS, SEQ, HW)[:, : SEQ - 1, :]
        .reshape(BS, SEQ - 1, NT, 128)
        .transpose(0, 3, 1, 2)
    )
    maskf = seq_mask.astype(np.float32)

    in_maps = []
    for b in range(BS):
        in_maps.append(
            {
                "k": ks[b].reshape(SEQ, HW, CK),
                "kb": kb[b].reshape(SEQ, HW, CK),
                "v": v[b].reshape(SEQ, HW, CV),
                "gate": gate_b[b].reshape(SEQ, HW),
                "gnat": gnat[b],
                "maskf": np.ascontiguousarray(maskf[b : b + 1]),
            }
        )
    return in_maps


def kernel(k, v, attention, seq_mask):
    v = np.ascontiguousarray(np.asarray(v, dtype=np.float32))

    if "nc" not in _CACHE:
        _CACHE["nc"] = build_program()
    nc = _CACHE["nc"]

    in_maps = make_in_maps(k, v, attention, seq_mask)
    res = run_bass_kernel_spmd(nc, in_maps, list(range(BS)))
    out_v = np.stack([res.results[b]["out_v"] for b in range(BS)]).reshape(
        BS, SEQ - 1, H, W, CV
    )
    gt = v[:, 1:].reshape(BS, SEQ - 1, H, W, CV)
    return out_v, gt


# revision 25
# speedup vs baseline: 1.0871x; 1.0042x over previous
"""KAST scatter-memory kernel for Trainium2 (8 NeuronCores, data-parallel over batch).

Per core: one batch element, 15 sequential steps. Design:
  - base-2 softmax: k pre-scaled by sqrt(log2 e) on host, E = 2^(L2 - C2)
  - k-branch logits in f32r (exact); f32 kT via PE transposes in a dedicated
    1-bank PSUM staging area, emitted as filler work between logit tiles
  - m-branch fully bf16: kT via DMA-transpose of host-supplied bf16 k,
    m_k EMA in bf16 on DVE (2x mode)
  - exp split: most kk-tiles on ACT (Exp with scale=ln2, bias=-C2*ln2 -> bf16 E),
    CHAIN_T tiles via DVE shift (L2-C2) + gpsimd pow(2, .) to offload ACT
  - rec reduction reoriented: out [128q, 4] per (sim, q-tile), E tile stationary
    -> near-zero PE cost, naturally-laid-out output; interleaved as PE filler
  - gates: host-computed sigmoid (no per-step ACT table swaps)
"""
import sys

sys.path.insert(0, "/opt/trn_rl_repo")

import numpy as np
import ml_dtypes

import concourse.bass as bass
import concourse.tile as tile
from concourse import bacc, mybir
from concourse.bass_utils import run_bass_kernel_spmd
from concourse.masks import make_identity

F32 = mybir.dt.float32
F32R = mybir.dt.float32r
BF16 = mybir.dt.bfloat16
AF = mybir.ActivationFunctionType
ALU = mybir.AluOpType

BS, SEQ, H, W, CK = 8, 16, 32, 32, 256
HW = H * W          # 1024
CV = 3
NT = HW // 128      # 8 hw tiles
NC2 = CK // 128     # 2 ck chunks
LN2 = 0.6931471805599453
SQRT_LOG2E = 1.2011224087864498   # k pre-scale: L2 = log2(e) * L
C2 = 80.0                         # base-2 shift: scaled logits <= ~196, rowmax >= ~33
CHAIN_T = (0, 4)                  # kk-tiles exp'd via DVE shift + gpsimd pow (per sim)
COEF = 0.1

_CACHE = {}


def _r(x):
    return x.bitcast(F32R)


def build_program():
    nc = bacc.Bacc("TRN2", target_bir_lowering=False, debug=False, num_devices=8)

    k_d = nc.dram_tensor("k", [SEQ, HW, CK], F32, kind="ExternalInput")  # * sqrt(log2 e)
    kb_d = nc.dram_tensor("kb", [SEQ, HW, CK], BF16, kind="ExternalInput")  # bf16 copy
    v_d = nc.dram_tensor("v", [SEQ, HW, CV], F32, kind="ExternalInput")
    g_d = nc.dram_tensor("gate", [SEQ, HW], BF16, kind="ExternalInput")  # sigmoid(att)
    gn_d = nc.dram_tensor("gnat", [128, SEQ - 1, NT], F32, kind="ExternalInput")
    m_d = nc.dram_tensor("maskf", [1, SEQ], F32, kind="ExternalInput")
    o_d = nc.dram_tensor("out_v", [SEQ - 1, HW, CV], F32, kind="ExternalOutput")

    with tile.TileContext(nc) as tc:
        with (
            tc.tile_pool(name="persist", bufs=1) as P1,
            tc.tile_pool(name="kn", bufs=4) as PKN,
            tc.tile_pool(name="knb", bufs=2) as PKNB,
            tc.tile_pool(name="kt", bufs=3) as PKT,
            tc.tile_pool(name="ktm", bufs=3) as PKTM,
            tc.tile_pool(name="ema", bufs=2) as PEMA,
            tc.tile_pool(name="g", bufs=2) as PG,
            tc.tile_pool(name="sh", bufs=4) as PSH,
            tc.tile_pool(name="ek", bufs=17) as PEK,
            tc.tile_pool(name="em", bufs=17) as PEM,
            tc.tile_pool(name="small", bufs=3) as PSM,
            tc.tile_pool(name="psL", bufs=3, space="PSUM") as PSL,
            tc.tile_pool(name="psT", bufs=1, space="PSUM") as PST,
            tc.tile_pool(name="psR", bufs=1, space="PSUM") as PSR,
        ):
            def transp_load(f):
                """DMAs for frame f: f32 kn in 2 ck-halves + bf16 DMA-transpose.
                Returns (kn_halves, kT tile to fill, kTm)."""
                knh = []
                for c in range(NC2):
                    kn = PKN.tile([128, NT, 128], F32, tag=f"kn{c}", name=f"kn{f}_{c}")
                    nc.sync.dma_start(
                        out=kn,
                        in_=k_d[f, :, c * 128 : (c + 1) * 128].rearrange(
                            "(t p) c -> p t c", p=128
                        ),
                    )
                    knh.append(kn)
                knb = PKNB.tile([128, NT, CK], BF16, tag="knb", name=f"knb{f}")
                nc.sync.dma_start(
                    out=knb, in_=kb_d[f].rearrange("(t p) c -> p t c", p=128)
                )
                # [128, 2t+s, 128]: kTm[p, 2t+s, i] = k_bf[t*128+i, s*128+p]
                kTm = PKTM.tile([128, 2 * NT, 128], BF16, tag="kTm", name=f"kTm{f}")
                nc.sync.dma_start_transpose(
                    out=kTm, in_=knb.rearrange("p t c -> p (t c)")
                )
                kT = PKT.tile([128, NC2, HW], F32, tag="kT", name=f"kT{f}")
                return knh, kT, kTm

            def tq_emit(f, knh, kT, q):
                """PE-transpose quarter q (c=q//2, half=q%2) of frame f into kT."""
                c, h = q // 2, q % 2
                ps = PST.tile([128, 512], F32, tag="pst", name=f"pst{f}_{q}")
                for tq in range(4):
                    nc.tensor.transpose(
                        out=ps[:, tq * 128 : (tq + 1) * 128],
                        in_=knh[c][:, h * 4 + tq, :],
                        identity=ident,
                    )
                nc.vector.tensor_copy(
                    out=_r(kT[:, c, h * 512 : (h + 1) * 512]), in_=ps
                )

            # --- early input DMAs for the first two frames
            lm0 = transp_load(0)
            lm1 = transp_load(1)

            ident = P1.tile([128, 128], F32)
            make_identity(nc, ident)
            negC2 = P1.tile([128, 1], F32)
            nc.vector.memset(negC2, -C2 * LN2)
            two = P1.tile([128, 1], F32)
            nc.vector.memset(two, 2.0)

            # persistent state
            m_kT = P1.tile([128, NC2, HW], BF16, tag="mkT")
            nc.gpsimd.memset(m_kT, 0.0)
            mv4 = P1.tile([128, 4 * NT], F32, tag="mv4")
            nc.vector.memset(mv4, 0.0)
            nc.vector.memset(mv4[:, 3 : 4 * NT : 4], 1.0)

            mask_bc = P1.tile([128, SEQ], F32, tag="maskbc")
            nc.sync.dma_start(out=mask_bc, in_=m_d[0:1, :].partition_broadcast(128))
            gnat = P1.tile([128, SEQ - 1, NT], F32, tag="gnat")
            nc.sync.dma_start(out=gnat, in_=gn_d[:, :, :])

            def load_v1(f):
                t = PSM.tile([128, NT, 4], F32, tag="v1")
                nc.vector.memset(t[:, :, 3:4], 1.0)
                nc.sync.dma_start(
                    out=t[:, :, 0:CV],
                    in_=v_d[f].rearrange("(t p) c -> p t c", p=128),
                )
                return t.rearrange("p t c -> p (t c)")

            # prologue: build kT(0), kT(1) immediately
            for q in range(4):
                tq_emit(0, lm0[0], lm0[1], q)
            for q in range(4):
                tq_emit(1, lm1[0], lm1[1], q)
            kT_i, kTm_i = lm0[1], lm0[2]
            kT_n, kTm_n = lm1[1], lm1[2]

            pv4raw = load_v1(0)
            pv4 = PSM.tile([128, 4 * NT], F32, tag="pv4", name="pv4_init")
            nc.vector.tensor_copy(out=_r(pv4), in_=pv4raw)
            pv4b = PSM.tile([128, 4 * NT], BF16, tag="pv4b", name="pv4b_init")
            nc.vector.tensor_copy(out=pv4b, in_=pv4)

            def logits_exp(i, sim, kT_st, kT_q, kTm_q, fillers):
                """Logits matmuls + exp for one sim; one filler popped per tile."""
                pool = PEK if sim == "k" else PEM
                E = [pool.tile([128, HW], BF16, tag="e" + sim, name=f"e{sim}{i}_{t}") for t in range(NT)]
                for t in range(NT):
                    ps = PSL.tile([128, HW], F32, tag="psl", name=f"psl{i}_{sim}{t}")
                    for c in range(NC2):
                        for hh in range(2):
                            if sim == "k":
                                nc.tensor.matmul(
                                    ps[:, hh * 512 : (hh + 1) * 512],
                                    _r(kT_st[:, c, t * 128 : (t + 1) * 128]),
                                    _r(kT_q[:, c, hh * 512 : (hh + 1) * 512]),
                                    start=(c == 0),
                                    stop=(c == NC2 - 1),
                                )
                            else:
                                nc.tensor.matmul(
                                    ps[:, hh * 512 : (hh + 1) * 512],
                                    m_kT[:, c, t * 128 : (t + 1) * 128],
                                    kTm_q[:, 8 * hh + c : 8 * hh + 8 : 2, :],
                                    start=(c == 0),
                                    stop=(c == NC2 - 1),
                                )
                    chain = t in CHAIN_T or (
                        i == SEQ - 2 and sim == "m" and t in (6, 7)
                    )
                    if chain:
                        sh = PSH.tile([128, HW], F32, tag="sh")
                        nc.vector.tensor_scalar_add(sh, ps, -C2)
                        nc.gpsimd.tensor_tensor(
                            out=E[t],
                            in0=two.broadcast_to([128, HW]),
                            in1=sh,
                            op=ALU.pow,
                        )
                    else:
                        nc.scalar.activation(
                            E[t], ps, AF.Exp, bias=negC2[:, 0:1], scale=LN2
                        )
                    if fillers:
                        fillers.pop(0)()
                return E

            def rec_chunks(j, E_k, E_m, pv4b_j, mv4b_j):
                """Return (recPS, 16 filler callables) for step j's reduction."""
                recPS = PSR.tile([128, 512], F32, tag="rec", name=f"rec{j}")
                fillers = []
                for s, (E, rhs4) in enumerate(((E_k, pv4b_j), (E_m, mv4b_j))):
                    for qt in range(NT):
                        def chunk(s=s, qt=qt, E=E, rhs4=rhs4):
                            for t in range(NT):
                                nc.tensor.matmul(
                                    recPS[:, s * 32 + qt * 4 : s * 32 + (qt + 1) * 4],
                                    E[t][:, qt * 128 : (qt + 1) * 128],
                                    rhs4[:, t * 4 : (t + 1) * 4],
                                    start=(t == 0),
                                    stop=(t == NT - 1),
                                    skip_group_check=True,
                                )
                        fillers.append(chunk)
                return recPS, fillers

            def blend_out(j, recPS):
                """rec = 0.9*Nk/Dk + 0.1*Nm/Dm from the accumulated bank; DMA out."""
                Nnat = PSM.tile([128, 64], F32, tag="Nnat")
                nc.vector.tensor_copy(out=Nnat, in_=recPS[:, 0:64])
                rD = PSM.tile([128, 16], F32, tag="rD")
                nc.vector.reciprocal(rD, Nnat[:, 3:64:4])
                nc.vector.tensor_scalar_mul(rD[:, 0:8], rD[:, 0:8], 1.0 - COEF)
                nc.vector.tensor_scalar_mul(rD[:, 8:16], rD[:, 8:16], COEF)
                rDe = PSM.tile([128, 16, 4], F32, tag="rDe")
                nc.vector.tensor_copy(
                    out=rDe, in_=rD.unsqueeze(-1).broadcast_to([128, 16, 4])
                )
                rDe = rDe.rearrange("p t c -> p (t c)")
                Ns = PSM.tile([128, 64], F32, tag="Ns")
                nc.vector.tensor_mul(Ns, Nnat, rDe)
                rec = PSM.tile([128, 32], F32, tag="recn")
                nc.vector.tensor_add(rec, Ns[:, 0:32], Ns[:, 32:64])
                nc.sync.dma_start(
                    out=o_d[j].rearrange("(t p) c -> p t c", p=128),
                    in_=rec.rearrange("p (t c) -> p t c", c=4)[:, :, 0:CV],
                )
                return rec

            prev = None  # (j, E_k, E_m, pv4b_j, mv4b_j) awaiting reduction
            for i in range(SEQ - 1):
                if i + 2 <= SEQ - 1:
                    kn2h, kT2, kTm_n2 = transp_load(i + 2)
                else:
                    kn2h, kT2, kTm_n2 = None, None, None

                # --- bf16 gate broadcast + m_kT EMA
                G = PG.tile([128, HW], BF16, tag="G")
                nc.sync.dma_start(
                    out=G, in_=g_d[i : i + 1, :].partition_broadcast(128)
                )
                for s in range(NC2):
                    ksv = kTm_i[:, s : 2 * NT : 2, :]                       # [128, 8t, 128]
                    msv = m_kT[:, s, :].rearrange("p (t i) -> p t i", i=128)
                    gsv = G.rearrange("p (t i) -> p t i", i=128)
                    tmp = PEMA.tile([128, NT, 128], BF16, tag=f"ema{s}")
                    nc.vector.tensor_sub(tmp, ksv, msv)
                    nc.vector.tensor_mul(tmp, tmp, gsv)
                    nc.vector.tensor_add(msv.bitcast(BF16), msv, tmp)

                # --- fillers: prev-step rec chunks + next-frame transposes
                if prev is not None:
                    j, pE_k, pE_m, ppv4b, pmv4b = prev
                    recPS, fillers = rec_chunks(j, pE_k, pE_m, ppv4b, pmv4b)
                else:
                    recPS, fillers = None, []
                if kn2h is not None:
                    for qi, pos in enumerate((2, 6, 10, 14)):
                        tq = (lambda q: lambda: tq_emit(i + 2, kn2h, kT2, q))(qi)
                        if pos < len(fillers):
                            fillers.insert(pos, tq)
                        else:
                            fillers.append(tq)

                # --- logits + exp for both sims, fillers interleaved
                E_k = logits_exp(i, "k", kT_i, kT_n, kTm_n, fillers)
                E_m = logits_exp(i, "m", kT_i, kT_n, kTm_n, fillers)
                for f in fillers:
                    f()

                # --- blend + output of step i-1 -> pv4(i)
                if prev is not None:
                    rec = blend_out(j, recPS)
                    v1 = load_v1(j)
                    diff = PSM.tile([128, 32], F32, tag="diff")
                    nc.vector.tensor_sub(diff, v1, rec)
                    nc.vector.tensor_scalar_mul(diff, diff, mask_bc[:, j : j + 1])
                    pv4_new = PSM.tile([128, 32], F32, tag="pv4")
                    nc.vector.tensor_add(_r(pv4_new), rec, diff)
                    pv4 = pv4_new
                    pv4b = PSM.tile([128, 32], BF16, tag="pv4b")
                    nc.vector.tensor_copy(out=pv4b, in_=pv4)

                # --- m_v EMA (pv of step i; only needed by recT(i) next iter)
                gb32 = PSM.tile([128, NT, 4], F32, tag="gb32")
                nc.vector.tensor_copy(
                    out=gb32,
                    in_=gnat[:, i, :].unsqueeze(-1).broadcast_to([128, NT, 4]),
                )
                gb32 = gb32.rearrange("p t c -> p (t c)")
                tmpv = PSM.tile([128, 4 * NT], F32, tag="tmpv")
                nc.vector.tensor_sub(tmpv, pv4, mv4)
                nc.vector.tensor_mul(tmpv, tmpv, gb32)
                nc.vector.tensor_add(_r(mv4), mv4, tmpv)
                mv4b = PSM.tile([128, 4 * NT], BF16, tag="mv4b")
                nc.vector.tensor_copy(out=mv4b, in_=mv4)

                prev = (i, E_k, E_m, pv4b, mv4b)
                if i < SEQ - 2:
                    kT_i, kTm_i = kT_n, kTm_n
                    kT_n, kTm_n = kT2, kTm_n2

            # epilogue: reduce the final step
            j, pE_k, pE_m, ppv4b, pmv4b = prev
            recPS, fillers = rec_chunks(j, pE_k, pE_m, ppv4b, pmv4b)
            for f in fillers:
                f()
            blend_out(j, recPS)

    nc.compile()
    return nc


def make_in_maps(k, v, attention, seq_mask):
    k = np.asarray(k, dtype=np.float32)
    v = np.ascontiguousarray(np.asarray(v, dtype=np.float32))
    attention = np.asarray(attention, dtype=np.float32)
    seq_mask = np.asarray(seq_mask)

    ks = np.ascontiguousarray(k * np.float32(SQRT_LOG2E))
    kb = np.ascontiguousarray(ks.astype(ml_dtypes.bfloat16))
    gate64 = 1.0 / (1.0 + np.exp(-attention, dtype=np.float64))
    gate = gate64.astype(np.float32)
    gate_b = np.ascontiguousarray(gate.astype(ml_dtypes.bfloat16))
    gnat = np.ascontiguousarray(
        gate.reshape(BS, SEQ, HW)[:, : SEQ - 1, :]
        .reshape(BS, SEQ - 1, NT, 128)
        .transpose(0, 3, 1, 2)
    )
    maskf = seq_mask.astype(np.float32)

    in_maps = []
    for b in range(BS):
        in_maps.append(
            {
                "k": ks[b].reshape(SEQ, HW, CK),
                "kb": kb[b].reshape(SEQ, HW, CK),
                "v": v[b].reshape(SEQ, HW, CV),
                "gate": gate_b[b].reshape(SEQ, HW),
                "gnat": gnat[b],
                "maskf": np.ascontiguousarray(maskf[b : b + 1]),
            }
        )
    return in_maps


def kernel(k, v, attention, seq_mask):
    v = np.ascontiguousarray(np.asarray(v, dtype=np.float32))

    if "nc" not in _CACHE:
        _CACHE["nc"] = build_program()
    nc = _CACHE["nc"]

    in_maps = make_in_maps(k, v, attention, seq_mask)
    res = run_bass_kernel_spmd(nc, in_maps, list(range(BS)))
    out_v = np.stack([res.results[b]["out_v"] for b in range(BS)]).reshape(
        BS, SEQ - 1, H, W, CV
    )
    gt = v[:, 1:].reshape(BS, SEQ - 1, H, W, CV)
    return out_v, gt
